# revision 1
# baseline (speedup 1.0000x reference)
"""Trainium2 Bass kernel for nn_Attention_15539191677265.

Single-head-dim attention block:
    qkv = w_qkv @ x ; per-head scaled dot-product attention over w=2048;
    out = w_out @ attn_out + b_out

Sharding: pure data-parallel over batch (b=8 -> 8 NeuronCores, one batch
element per core). Weights are replicated. No collectives.

Per-core algorithm (transposed-softmax scheme, all matmuls bf16):
  1. q,k = wqkvT.T @ x           ([c,o] stationary; q pre-scaled on host)
  2. vT  = x.T @ wvT             (v produced directly transposed [j, d])
  3. per head: sim^T[j,i] strips -> exp on ScalarE (no max subtraction:
     scores are ~N(0,1), exp cannot overflow in fp32/bf16 range)
  4. AV: out^T[d,i] = vT.T @ exp_strip, with a ones-column appended to vT
     so row 64 of the psum accumulates the softmax normalizer for free
  5. normalize: reciprocal(norm row) -> partition_broadcast -> multiply
  6. proj: out = woutT.T @ attn_out (per-head K=64 chunks) + bias
"""

import sys

if "/opt/trn_rl_repo" not in sys.path:
    sys.path.insert(0, "/opt/trn_rl_repo")

import numpy as np
import ml_dtypes

import concourse.bass as bass
import concourse.mybir as mybir
import concourse.tile as tile
from concourse import bacc
from concourse.bass_utils import run_bass_kernel_spmd

BF16 = mybir.dt.bfloat16
F32 = mybir.dt.float32
EXP = mybir.ActivationFunctionType.Exp

B, DIM, W = 8, 256, 2048
HEADS, DH = 8, 64
HID = HEADS * DH  # 512
SCALE = DH ** (-0.5)
N_CORES = 8

NJT = W // 128  # 16 j-tiles per head
NCT = DIM // 128  # 2 contraction chunks over channels


def build_kernel():
    nc = bacc.Bacc(None, target_bir_lowering=False)

    x_d = nc.dram_tensor("x", [DIM, W], BF16, kind="ExternalInput")
    wqkvT_d = nc.dram_tensor("wqkvT", [DIM, 3 * HID], BF16, kind="ExternalInput")
    woutT_d = nc.dram_tensor("woutT", [128, 4, DIM], BF16, kind="ExternalInput")
    bias_d = nc.dram_tensor("bias", [128, DIM // 128], F32, kind="ExternalInput")
    out_d = nc.dram_tensor("out", [DIM, W], F32, kind="ExternalOutput")

    with tile.TileContext(nc) as tc:
        with tc.tile_pool(name="pers", bufs=1) as pers:
            x_sb = pers.tile([128, NCT, W], BF16)
            wq_sb = pers.tile([128, NCT, 3 * HID], BF16)
            wo_sb = pers.tile([128, 4, DIM], BF16)
            bias_sb = pers.tile([128, DIM // 128], F32)
            q_sb = pers.tile([128, 4, W], BF16)
            k_sb = pers.tile([128, 4, W], BF16)
            vt_sb = pers.tile([128, NJT, HEADS, 128], BF16)
            attout_sb = [
                pers.tile([128, W], BF16, name=f"attout_{kc}", tag=f"attout{kc}")
                for kc in range(4)
            ]
            out_sb = pers.tile([128, NCT, W], F32)

            xr = x_d[:].rearrange("(ct p) w -> p ct w", p=128)
            for ct in range(NCT):
                for wh in range(4):
                    nc.sync.dma_start(
                        out=x_sb[:, ct, wh * 512 : (wh + 1) * 512],
                        in_=xr[:, ct, wh * 512 : (wh + 1) * 512],
                    )
            wqr = wqkvT_d[:].rearrange("(ct p) o -> p ct o", p=128)
            for ct in range(NCT):
                for sec in range(3):
                    nc.sync.dma_start(
                        out=wq_sb[:, ct, sec * HID : (sec + 1) * HID],
                        in_=wqr[:, ct, sec * HID : (sec + 1) * HID],
                    )
            nc.sync.dma_start(out=wo_sb[:], in_=woutT_d[:])
            nc.sync.dma_start(out=bias_sb[:], in_=bias_d[:])

            # cols 64..127: ones column then zero padding (FWL needs 128)
            nc.vector.memset(vt_sb[:, :, :, DH:128], 0.0)
            nc.vector.memset(vt_sb[:, :, :, DH : DH + 1], 1.0)
            # warm the ACT exp table set while qkv matmuls run
            warm = pers.tile([1, 1], F32)
            nc.vector.memset(warm[:], 0.0)
            nc.scalar.activation(out=warm[:], in_=warm[:], func=EXP)

            # ---- phase 1: q, k projections -> [128, 4, W] bf16 each ----
            # [128,1024] psum pieces with 4-slot rotation: evacuation copies
            # (alternating DVE/ACT) never stall the o-tile matmul stream
            with tc.tile_pool(name="qkv_ps", bufs=8, space="PSUM") as qkv_ps:
                for ot in range(8):  # o-tiles 0..3 = q, 4..7 = k
                    dst = q_sb if ot < 4 else k_sb
                    for ph in range(4):
                        po = ph * 512
                        ps = qkv_ps.tile(
                            [128, 512], F32, name=f"qk_{ot}_{ph}", tag="qk"
                        )
                        for ct in range(NCT):
                            nc.tensor.matmul(
                                ps[:],
                                lhsT=wq_sb[:, ct, ot * 128 : (ot + 1) * 128],
                                rhs=x_sb[:, ct, po : po + 512],
                                start=(ct == 0),
                                stop=(ct == NCT - 1),
                            )
                        if (4 * ot + ph) % 2 == 0:
                            nc.vector.tensor_copy(
                                out=dst[:, ot % 4, po : po + 512], in_=ps[:]
                            )
                        else:
                            nc.scalar.copy(
                                out=dst[:, ot % 4, po : po + 512], in_=ps[:]
                            )

            # ---- phase 2: vT[j, hd] = x.T @ wvT  (+ones col kept) ----
            with tc.tile_pool(name="vt_ps", bufs=8, space="PSUM") as vt_ps:
                for jt in range(NJT):
                    ps = vt_ps.tile([128, HID], F32)
                    for ct in range(NCT):
                        nc.tensor.matmul(
                            ps[:],
                            lhsT=x_sb[:, ct, jt * 128 : (jt + 1) * 128],
                            rhs=wq_sb[:, ct, 2 * HID : 3 * HID],
                            start=(ct == 0),
                            stop=(ct == NCT - 1),
                        )
                    if jt % 2 == 0:
                        nc.vector.tensor_copy(
                            out=vt_sb[:, jt, :, 0:DH],
                            in_=ps[:].rearrange("p (h d) -> p h d", h=HEADS),
                        )
                    else:
                        nc.scalar.copy(
                            out=vt_sb[:, jt, :, 0:DH],
                            in_=ps[:].rearrange("p (h d) -> p h d", h=HEADS),
                        )

            # ---- phase 3: attention per head ----
            with (
                tc.tile_pool(name="strip_ps", bufs=1, space="PSUM") as strip_ps,
                tc.tile_pool(name="av_ps", bufs=2, space="PSUM") as av_ps,
                tc.tile_pool(name="exp_sb", bufs=10) as exp_pool,
                tc.tile_pool(name="norm_sb", bufs=4) as norm_pool,
            ):
                for h in range(HEADS):
                    kt, koff = h // 2, (h % 2) * 64
                    avs = [
                        av_ps.tile([128, 1024], F32, name=f"av_{h}_{ih}", tag="av")
                        for ih in range(2)
                    ]
                    for jt in range(NJT):
                        # 4 sim matmuls sharing one k-tile LDWEIGHTS, then
                        # 4 AV matmuls sharing one vT LDWEIGHTS
                        strips = []
                        for ih in range(2):
                            io = ih * 1024
                            strip = strip_ps.tile(
                                [128, 1024], F32, name=f"st_{ih}", tag=f"st{ih}"
                            )
                            strips.append(strip)
                            for ns in range(2):
                                nc.tensor.matmul(
                                    strip[:, ns * 512 : (ns + 1) * 512],
                                    lhsT=k_sb[
                                        koff : koff + DH, kt, jt * 128 : (jt + 1) * 128
                                    ],
                                    rhs=q_sb[
                                        koff : koff + DH,
                                        kt,
                                        io + ns * 512 : io + (ns + 1) * 512,
                                    ],
                                    start=True,
                                    stop=True,
                                )
                        ess = []
                        for ih in range(2):
                            es = exp_pool.tile(
                                [128, 1024], BF16, name=f"es_{ih}", tag="es"
                            )
                            ess.append(es)
                            nc.scalar.activation(
                                out=es[:], in_=strips[ih][:], func=EXP
                            )
                        for ih in range(2):
                            for ns in range(2):
                                nc.tensor.matmul(
                                    avs[ih][:, ns * 512 : (ns + 1) * 512],
                                    lhsT=vt_sb[:, jt, h, :],
                                    rhs=ess[ih][:, ns * 512 : (ns + 1) * 512],
                                    start=(jt == 0),
                                    stop=(jt == NJT - 1),
                                )
                    for ih in range(2):
                        io = ih * 1024
                        av = avs[ih]
                        # evacuate psum right away so the av slot frees for the
                        # next head; the norm chain then runs off-critical-path
                        avc = norm_pool.tile([DH + 1, 1024], F32, tag="avc")
                        nc.vector.tensor_copy(out=avc[:], in_=av[0 : DH + 1, :])
                        rec0 = norm_pool.tile([1, 1024], F32, tag="rec0")
                        bcn = norm_pool.tile([DH, 1024], F32, tag="bcn")
                        bc = norm_pool.tile([DH, 1024], F32, tag="bc")
                        # partition_broadcast + custom-DVE ops only work from
                        # partition 0 -> DMA the raw norm row there first
                        nc.sync.dma_start(out=rec0[:], in_=avc[DH : DH + 1, :])
                        nc.gpsimd.partition_broadcast(
                            bcn[:], rec0[0:1, :], channels=DH
                        )
                        nc.vector.reciprocal_approx_fast(out=bc[:], in_=bcn[:])
                        if h % 2 == 0:
                            nc.vector.tensor_mul(
                                out=attout_sb[h // 2][0:DH, io : io + 1024],
                                in0=avc[0:DH, :],
                                in1=bc[:],
                            )
                        else:
                            # odd heads land on partitions 64..127: DVE cannot
                            # shift partitions, so write via a bounce + DMA
                            atmp = norm_pool.tile([DH, 1024], BF16, tag="atmp")
                            nc.vector.tensor_mul(
                                out=atmp[:], in0=avc[0:DH, :], in1=bc[:]
                            )
                            nc.sync.dma_start(
                                out=attout_sb[h // 2][DH:128, io : io + 1024],
                                in_=atmp[:],
                            )

            # ---- phase 4: output projection + bias (K=128 head pairs) ----
            outr = out_d[:].rearrange("(ct p) w -> p ct w", p=128)
            with tc.tile_pool(name="proj_ps", bufs=8, space="PSUM") as proj_ps:
                for ot in range(NCT):
                    for wh in range(4):
                        wo = wh * 512
                        ps = proj_ps.tile(
                            [128, 512], F32, name=f"pj_{ot}_{wh}", tag="pj"
                        )
                        for kc in range(4):
                            nc.tensor.matmul(
                                ps[:],
                                lhsT=wo_sb[:, kc, ot * 128 : (ot + 1) * 128],
                                rhs=attout_sb[kc][:, wo : wo + 512],
                                start=(kc == 0),
                                stop=(kc == 3),
                            )
                        nc.vector.tensor_scalar_add(
                            out=out_sb[:, ot, wo : wo + 512],
                            in0=ps[:],
                            scalar1=bias_sb[:, ot : ot + 1],
                        )
                        nc.sync.dma_start(
                            out=outr[:, ot, wo : wo + 512],
                            in_=out_sb[:, ot, wo : wo + 512],
                        )

    nc.compile()
    return nc



_NC_CACHE = None


def _get_nc():
    global _NC_CACHE
    if _NC_CACHE is None:
        _NC_CACHE = build_kernel()
    return _NC_CACHE


def make_in_maps(x, w_qkv, w_out, b_out):
    bf16 = ml_dtypes.bfloat16
    wq = np.array(w_qkv, dtype=np.float32, copy=True)
    wq[:HID] *= SCALE  # fold attention scale into the q projection
    wqkvT = np.ascontiguousarray(wq.T).astype(bf16)  # [256, 1536]
    woutT = np.ascontiguousarray(
        w_out.T.reshape(4, 128, DIM).transpose(1, 0, 2)
    ).astype(bf16)  # [128, 4, 256]
    bias = np.ascontiguousarray(
        b_out.astype(np.float32).reshape(DIM // 128, 128).T
    )  # [128, 2]
    in_maps = []
    for i in range(N_CORES):
        in_maps.append(
            {
                "x": x[i].astype(bf16),
                "wqkvT": wqkvT,
                "woutT": woutT,
                "bias": bias,
            }
        )
    return in_maps


def kernel(x, w_qkv, w_out, b_out, _trace=False):
    nc = _get_nc()
    in_maps = make_in_maps(x, w_qkv, w_out, b_out)
    res = run_bass_kernel_spmd(
        nc,
        in_maps,
        core_ids=list(range(N_CORES)),
        trace=_trace,
        trace_cores=list(range(N_CORES)) if _trace else None,
    )
    out = np.stack([res.results[i]["out"] for i in range(N_CORES)], axis=0)
    if _trace:
        kernel.last_exec_time_ns = res.exec_time_ns
        kernel.last_results = res
    return out



# revision 2
# speedup vs baseline: 1.1833x; 1.1833x over previous
"""Trainium2 Bass kernel for nn_Attention_15539191677265 (v3).

Sharding: pure data-parallel over batch (b=8 -> 8 NeuronCores). No collectives.

Per-core algorithm (everything inside the head loop is fp8 DoubleRow at
K=256 logical contraction -> 1 cycle per output column; bf16 K=64 matmuls
run at only 0.5 col/cycle on TRN2 so they are avoided entirely):

  1. q,k,v projections in bf16 (K=128, full rate). q pre-scaled on host.
  2. q,k split hi+lo e4m3 (DVE casts from psum), then assembled via
     sbuf->sbuf DMAs (free partition moves) into DoubleRow operand layouts:
       q_hl [128=(q_hi d | q_lo d), head, slot2(dup), W]
       khl  [128=(dup d), head, jt, slot2=(k_hi|k_lo), 128]
     sim = (q_hi+q_lo)x(k_hi+k_lo) exactly: 4-way compensated, near-bf16
     accuracy at fp8 speed.
  3. sim strips [128 j, 2 jt, 512 i] psum; exp on ACT (one instr per strip
     pair, bias=ln(1/8) guards e4m3 overflow; sim absmax ~7.5, e4m3 max 448)
     writing e4m3 directly.
  4. AV: DoubleRow, contraction = j-tile pairs; stationary
     [v_hi(64) | ones(64) | v_lo(65:)] so psum row 64 accumulates the
     softmax normalizer for free (v hi/lo compensated except d=63).
  5. normalize: evac chunk, recip row 64, broadcast, multiply (Pool).
  6. proj: w_out with host-duplicated hi/lo rows (K=8x128), bias, out f32.

ACT does only exp (33.5M exps/core is the wall); evacuations on DVE,
normalize multiply + broadcast on Pool (Pool cannot read PSUM).
PSUM accumulation note: a matmul's start flag resets the whole 2KB bank,
so the two 256-col DoubleRow chunks per bank get their own start=True
zero-writes at t=0 before any data lands, then accumulate with start=False.
"""

import sys

if "/opt/trn_rl_repo" not in sys.path:
    sys.path.insert(0, "/opt/trn_rl_repo")

import math

import numpy as np
import ml_dtypes

import concourse.bass as bass
import concourse.mybir as mybir
import concourse.tile as tile
from concourse import bacc
from concourse.bass_utils import run_bass_kernel_spmd

BF16 = mybir.dt.bfloat16
F8 = mybir.dt.float8e4
F32 = mybir.dt.float32
EXP = mybir.ActivationFunctionType.Exp
DR = mybir.MatmulPerfMode.DoubleRow

B, DIM, W = 8, 256, 2048
HEADS, DH = 8, 64
HID = HEADS * DH  # 512
SCALE = DH ** (-0.5)
N_CORES = 8

NJT = W // 128  # 16 j-tiles
NPAIR = NJT // 2  # 8 j-tile pairs
NCT = DIM // 128  # 2 contraction chunks over channels
LN_PSCALE = math.log(0.125)  # exp output scale, cancels in normalization


def build_kernel():
    nc = bacc.Bacc(None, target_bir_lowering=False)

    x_d = nc.dram_tensor("x", [DIM, W], BF16, kind="ExternalInput")
    wqkvT_d = nc.dram_tensor("wqkvT", [DIM, 3 * HID], BF16, kind="ExternalInput")
    woutT_d = nc.dram_tensor("woutT", [128, HEADS, DIM], BF16, kind="ExternalInput")
    bias_d = nc.dram_tensor("bias", [128, DIM // 128], F32, kind="ExternalInput")
    out_d = nc.dram_tensor("out", [DIM, W], F32, kind="ExternalOutput")

    with tile.TileContext(nc) as tc:
        with tc.tile_pool(name="pers", bufs=1) as pers:
            x_sb = pers.tile([128, NCT, W], BF16)
            wq_sb = pers.tile([128, NCT, 3 * HID], BF16)
            wo_sb = pers.tile([128, HEADS, DIM], BF16)
            bias_sb = pers.tile([128, DIM // 128], F32)
            # DoubleRow sim operands (slot-major over full W so the assembly
            # DMAs write contiguous per-partition runs - descriptor count!)
            q_hl = pers.tile([128, HEADS, 2, W], F8)  # 32KB/part
            khl = pers.tile([128, HEADS, 2, W], F8)  # 32KB/part
            # DoubleRow AV stationary [j, head, pair, slot, m]
            vhl_sb = pers.tile([128, HEADS, NPAIR, 2, 128], F8)
            attout_sb = [
                pers.tile([128, W], BF16, name=f"attout_{h}", tag=f"attout{h}")
                for h in range(HEADS)
            ]
            out_sb = pers.tile([128, NCT, W], F32)

            xr = x_d[:].rearrange("(ct p) w -> p ct w", p=128)
            for ct in range(NCT):
                for wh in range(2):
                    nc.sync.dma_start(
                        out=x_sb[:, ct, wh * 1024 : (wh + 1) * 1024],
                        in_=xr[:, ct, wh * 1024 : (wh + 1) * 1024],
                    )
            wqr = wqkvT_d[:].rearrange("(ct p) o -> p ct o", p=128)
            for ct in range(NCT):
                for sec in range(3):
                    nc.sync.dma_start(
                        out=wq_sb[:, ct, sec * HID : (sec + 1) * HID],
                        in_=wqr[:, ct, sec * HID : (sec + 1) * HID],
                    )
            nc.sync.dma_start(out=wo_sb[:], in_=woutT_d[:])
            nc.sync.dma_start(out=bias_sb[:], in_=bias_d[:])

            # AV stationary m-layout: [v_hi(0:64) | ones(64) | v_lo(65:128)]
            nc.vector.memset(vhl_sb[:, :, :, :, DH : DH + 1], 1.0)
            # warm the ACT exp table early
            warm = pers.tile([1, 1], F32)
            nc.vector.memset(warm[:], 0.0)
            nc.scalar.activation(out=warm[:], in_=warm[:], func=EXP)
            # per-partition bias AP holding ln(1/8) for the exp instructions
            pbias = pers.tile([128, 1], F32)
            nc.vector.memset(pbias[:], LN_PSCALE)

            # ---- phase 1a: q,k projections; hi/lo e4m3 staging + DMA
            # assembly into the DoubleRow layouts ----
            with (
                tc.tile_pool(name="qkv_ps", bufs=2, space="PSUM") as qkv_ps,
                tc.tile_pool(name="stage", bufs=4) as stage,
            ):
                for ot in range(8):  # interleaved per k-tile: q,k for kt0 first
                    is_q = ot % 2 == 0
                    kt = ot // 2
                    hi8 = stage.tile([128, W], F8, name=f"hi_{ot}", tag="hi")
                    lo8 = stage.tile([128, W], F8, name=f"lo_{ot}", tag="lo")
                    ps = qkv_ps.tile([128, W], F32, name=f"qk_{ot}", tag="qk")
                    for ph in range(4):
                        po = ph * 512
                        for ct in range(NCT):
                            nc.tensor.matmul(
                                ps[:, po : po + 512],
                                lhsT=wq_sb[
                                    :, ct, (HID if not is_q else 0) + kt * 128 :
                                    (HID if not is_q else 0) + (kt + 1) * 128
                                ],
                                rhs=x_sb[:, ct, po : po + 512],
                                start=(ct == 0),
                                stop=(ct == NCT - 1),
                            )
                    # hi cast on ACT (idle in phase 1), lo sub on DVE
                    nc.scalar.copy(out=hi8[:], in_=ps[:])
                    nc.vector.tensor_sub(out=lo8[:], in0=ps[:], in1=hi8[:])
                    # DMA-assemble the two heads of this k-tile
                    for hh in range(2):
                        h = 2 * kt + hh
                        koff = hh * 64
                        if is_q:
                            for s in range(2):
                                nc.sync.dma_start(
                                    out=q_hl[0:64, h, s, :],
                                    in_=hi8[koff : koff + 64, :],
                                )
                                nc.sync.dma_start(
                                    out=q_hl[64:128, h, s, :],
                                    in_=lo8[koff : koff + 64, :],
                                )
                        else:
                            for half in range(2):
                                pd = half * 64
                                nc.sync.dma_start(
                                    out=khl[pd : pd + 64, h, 0, :],
                                    in_=hi8[koff : koff + 64, :],
                                )
                                nc.sync.dma_start(
                                    out=khl[pd : pd + 64, h, 1, :],
                                    in_=lo8[koff : koff + 64, :],
                                )

            # ---- phase 1b: vT = x.T @ wvT, split hi/lo e4m3 ----
            # two j-tiles (one pair) per psum tile so the evac ops are 2x wider
            with tc.tile_pool(name="vt_ps", bufs=4, space="PSUM") as vt_ps:
                for t in range(NPAIR):
                    ps = vt_ps.tile([128, 2, HID], F32, name=f"vt_{t}", tag="vt")
                    for s in range(2):
                        jt = 2 * t + s
                        for ct in range(NCT):
                            nc.tensor.matmul(
                                ps[:, s, :],
                                lhsT=x_sb[:, ct, jt * 128 : (jt + 1) * 128],
                                rhs=wq_sb[:, ct, 2 * HID : 3 * HID],
                                start=(ct == 0),
                                stop=(ct == NCT - 1),
                            )
                    psr = ps[:].rearrange("p s (h d) -> p s h d", h=HEADS)
                    # dst [p, h, t, s, m]: h outer, s middle - swap via strides
                    dst_hi = vhl_sb[:, :, t, :, 0:DH].rearrange("p h s m -> p s h m")
                    dst_lo = vhl_sb[:, :, t, :, DH + 1 : 128].rearrange(
                        "p h s m -> p s h m"
                    )
                    in_lo = vhl_sb[:, :, t, :, 0:63].rearrange("p h s m -> p s h m")
                    nc.scalar.copy(out=dst_hi, in_=psr)
                    nc.vector.tensor_sub(
                        out=dst_lo, in0=psr[:, :, :, 0:63], in1=in_lo
                    )

            # ---- phase 2: attention per head, software-pipelined ----
            outr = out_d[:].rearrange("(ct p) w -> p ct w", p=128)
            with (
                tc.tile_pool(name="strip_ps", bufs=3, space="PSUM") as strip_ps,
                tc.tile_pool(name="av_ps", bufs=2, space="PSUM") as av_ps,
                tc.tile_pool(name="es_sb", bufs=4) as es_pool,
                tc.tile_pool(name="norm_sb", bufs=2) as norm_pool,
            ):
                NIT = 4 * NPAIR  # 32 iterations per head

                def sim_exp(h, idx, es_ring):
                    ipass, t = divmod(idx, NPAIR)
                    io = ipass * 512
                    strip = strip_ps.tile(
                        [128, 2, 512], F32, name=f"st_{h}_{idx}", tag="st"
                    )
                    for s in range(2):
                        jt = 2 * t + s
                        for c in range(2):
                            nc.tensor.matmul(
                                strip[:, s, c * 256 : (c + 1) * 256],
                                lhsT=khl[:, h, :, jt * 128 : (jt + 1) * 128],
                                rhs=q_hl[:, h, :, io + c * 256 : io + (c + 1) * 256],
                                start=True,
                                stop=True,
                                perf_mode=DR,
                                skip_group_check=True,
                            )
                    es = es_pool.tile(
                        [128, 2, 512], F8, name=f"es_{h}_{idx}", tag="es"
                    )
                    nc.scalar.activation(
                        out=es[:], in_=strip[:], func=EXP, bias=pbias[:]
                    )
                    es_ring[idx] = es

                def av_mm(h, idx, avs, es_ring):
                    ipass, t = divmod(idx, NPAIR)
                    av = avs["cur"]
                    es = es_ring.pop(idx)
                    for c in range(2):
                        nc.tensor.matmul(
                            av[:, c * 256 : (c + 1) * 256],
                            lhsT=vhl_sb[:, h, t, :, :],
                            rhs=es[:, :, c * 256 : (c + 1) * 256],
                            start=False,
                            stop=(t == NPAIR - 1),
                            perf_mode=DR,
                            skip_group_check=True,
                        )

                def zero_open(h, ipass, avs):
                    # open the bank with start=True zero-writes before any
                    # data lands (start resets the whole 2KB bank)
                    avs["cur"] = av = av_ps.tile(
                        [128, 512], F32, name=f"av_{h}_{ipass}", tag="av"
                    )
                    for c in range(2):
                        nc.tensor.matmul(
                            av[:, c * 256 : (c + 1) * 256],
                            lhsT=vhl_sb[:, 0, 0, :, 0:128],
                            rhs=zes[:, :, 0:256],
                            start=True,
                            stop=False,
                            perf_mode=DR,
                            skip_group_check=True,
                        )

                def normalize(h, ipass, avs):
                    # evac psum fast so the av bank frees for ipass+2
                    io = ipass * 512
                    av = avs["cur"]
                    avc = norm_pool.tile([128, 512], F32, tag="avc")
                    nc.vector.tensor_copy(out=avc[:], in_=av[:])
                    rec0 = norm_pool.tile([1, 512], F32, tag="rec0")
                    nc.sync.dma_start(out=rec0[:], in_=avc[DH : DH + 1, :])
                    rec = norm_pool.tile([1, 512], F32, tag="rec")
                    nc.vector.reciprocal_approx_fast(out=rec[:], in_=rec0[:])
                    bcn = norm_pool.tile([128, 512], F32, tag="bcn")
                    nc.gpsimd.partition_broadcast(bcn[:], rec[0:1, :], channels=128)
                    # keep gpsimd single-library (broadcast only): a second op
                    # kind would thrash the DSP library (~6us reload each)
                    nc.vector.tensor_mul(
                        out=attout_sb[h][:, io : io + 512],
                        in0=avc[:],
                        in1=bcn[:],
                    )

                # zero fp8 moving operand for the bank-opening matmuls
                zes = es_pool.tile([128, 2, 512], F8, name="zes", tag="zes")
                nc.vector.memset(zes[:], 0.0)

                es_ring = {}
                avs = {}
                for gi in range(HEADS * NIT + 1):
                    h, idx = divmod(gi, NIT)
                    if gi < HEADS * NIT:
                        sim_exp(h, idx, es_ring)
                    if gi >= 1:
                        ph_, pidx = divmod(gi - 1, NIT)
                        ipass, t = divmod(pidx, NPAIR)
                        if t == 0:
                            zero_open(ph_, ipass, avs)
                        av_mm(ph_, pidx, avs, es_ring)
                        if t == NPAIR - 1:
                            normalize(ph_, ipass, avs)

            # ---- phase 3: output projection + bias (K=8x128 hi/lo packed) ----
            with tc.tile_pool(name="proj_ps", bufs=8, space="PSUM") as proj_ps:
                for ot in range(NCT):
                    # kc-outer/wh-inner: each w_out stationary serves 4 chunks
                    pss = [
                        proj_ps.tile([128, 512], F32, name=f"pj_{ot}_{wh}", tag="pj")
                        for wh in range(4)
                    ]
                    for kc in range(HEADS):
                        for wh in range(4):
                            nc.tensor.matmul(
                                pss[wh][:],
                                lhsT=wo_sb[:, kc, ot * 128 : (ot + 1) * 128],
                                rhs=attout_sb[kc][:, wh * 512 : (wh + 1) * 512],
                                start=(kc == 0),
                                stop=(kc == HEADS - 1),
                            )
                    for wh in range(4):
                        wo = wh * 512
                        nc.vector.tensor_scalar_add(
                            out=out_sb[:, ot, wo : wo + 512],
                            in0=pss[wh][:],
                            scalar1=bias_sb[:, ot : ot + 1],
                        )
                        nc.sync.dma_start(
                            out=outr[:, ot, wo : wo + 512],
                            in_=out_sb[:, ot, wo : wo + 512],
                        )

    nc.compile()
    return nc


_NC_CACHE = None


def _get_nc():
    global _NC_CACHE
    if _NC_CACHE is None:
        _NC_CACHE = build_kernel()
    return _NC_CACHE


def make_in_maps(x, w_qkv, w_out, b_out):
    bf16 = ml_dtypes.bfloat16
    wq = np.array(w_qkv, dtype=np.float32, copy=True)
    wq[:HID] *= SCALE  # fold attention scale into the q projection
    wqkvT = np.ascontiguousarray(wq.T).astype(bf16)  # [256, 1536]
    # w_out rows per head: [d 0-63 | zero (norm row) | d 0-62 (lo)]
    wt = np.asarray(w_out, dtype=np.float32).T  # [512, 256]
    wo = np.zeros((128, HEADS, DIM), dtype=np.float32)
    for h in range(HEADS):
        wo[0:64, h, :] = wt[h * 64 : (h + 1) * 64, :]
        wo[65:128, h, :] = wt[h * 64 : h * 64 + 63, :]
    woutT = wo.astype(bf16)
    bias = np.ascontiguousarray(
        np.asarray(b_out, dtype=np.float32).reshape(DIM // 128, 128).T
    )  # [128, 2]
    in_maps = []
    for i in range(N_CORES):
        in_maps.append(
            {
                "x": np.asarray(x[i]).astype(bf16),
                "wqkvT": wqkvT,
                "woutT": woutT,
                "bias": bias,
            }
        )
    return in_maps


def kernel(x, w_qkv, w_out, b_out, _trace=False):
    nc = _get_nc()
    in_maps = make_in_maps(x, w_qkv, w_out, b_out)
    res = run_bass_kernel_spmd(
        nc,
        in_maps,
        core_ids=list(range(N_CORES)),
        trace=_trace,
        trace_cores=list(range(N_CORES)) if _trace else None,
    )
    out = np.stack([res.results[i]["out"] for i in range(N_CORES)], axis=0)
    if _trace:
        kernel.last_exec_time_ns = res.exec_time_ns
        kernel.last_results = res
    return out
